# revision 18
# baseline (speedup 1.0000x reference)
"""Multi-head self-attention (B=4, S=2048, D=1024, H=16) on 8 Trainium2 cores.

Sharding: core c handles batch c//2 and head-half c%2 (8 heads = 512 dims).
Each core computes q/k/v projections for its heads, attention, and a partial
output projection (contraction over its 512 head-dims). Host sums the two
partials per batch and adds bo.

Schedule (HW-A/B-tested against the previous versions):
- qk_proj(0) emits k-chains + the first q-chunk chain first; scores (and
  ScalarE exp) start after 2 chains instead of 8. q1..q3 stream later.
- The two head-par AV chains run kt-interleaved on the two AV PSUM banks
  (no same-bank back-to-back accumulation) and normalization reads each
  bank directly: no cross-bank merge adds on DVE (~40us less DVE busy).
Earlier findings kept:
- Coarse 3D prologue DMAs in compute-priority order (x + m0 columns of
  wq/wk first): the first projection chain starts ~9us earlier.
- q/k projection chains for head-pair m+1 spread 3/3/2 over the qc=0..2
  windows of m instead of one lump at qc==2 (no PE burst at m boundaries).
- v-projection chains interleaved one-per-score-unit inside (0,0) so the
  first AV chain is unblocked as early as possible; epool 25 bufs of
  backlog keep ScalarE fed across that window.
- m=3 AV drains immediately followed by that q-block's output projection,
  overlapping the output tail with the last attention windows.

Self-contained: hardcodes all shapes; only needs the environment-provided
concourse libraries.
"""
import os
import sys

for _p in ("/opt/trn_rl_repo", "/root/.axon_site/_ro/trn_rl_repo"):
    if os.path.isdir(_p) and _p not in sys.path:
        sys.path.insert(0, _p)

import numpy as np
import ml_dtypes

import concourse.bass as bass  # noqa: F401
from concourse import bacc
import concourse.mybir as mybir
import concourse.tile as tile
from concourse.bass_utils import run_bass_kernel_spmd

# problem constants
B, S, D, H, Hd = 4, 2048, 1024, 16, 64
NCORES = 8
HH = H // 2            # heads per core
DH = HH * Hd           # head dims per core = 512
SCALE = 1.0 / np.sqrt(Hd)

ST = S // 128          # 16 seq tiles
QC = S // 512          # 4 q chunks
KT = ST                # 16 k tiles
DT_IN = D // 128       # 8 input-dim tiles
MT = DH // 128         # 4 head-pair tiles

f32 = mybir.dt.float32
bf16 = mybir.dt.bfloat16
np_bf16 = ml_dtypes.bfloat16

# matmul input dtypes per stage (fp32 accumulation always).
# float16: 1 cy/row on PE like bf16, 8x finer mantissa. exp(logit) must stay
# below 65504 — max |logit| for randn-style inputs is ~9 (exp ~6.6e3), safe.
PROJ_DT = mybir.dt.float16    # xT / wq / wk / wv
QK_DT = mybir.dt.float16      # qT / kT (scores matmul operands)
E_DT = mybir.dt.float16       # exp outputs and v_aug (AV matmul operands)
VO_DT = mybir.dt.float16      # valsn / wo (output projection operands)
_NP = {
    mybir.dt.bfloat16: np_bf16,
    mybir.dt.float16: np.float16,
    mybir.dt.float32r: np.float32,
    mybir.dt.float32: np.float32,
}

_CACHE = {}

# timing-ablation knob: subset of {"scores", "exp", "av", "outproj", "norm"}
# suppresses those instruction groups (breaks semantics, timing only)
SKIP = set()


def _build_program(repeat=1):
    nc = bacc.Bacc("TRN2", target_bir_lowering=False, debug=False)

    xt_d = nc.declare_dram_parameter("xt", [D, S], PROJ_DT, isOutput=False)
    wq_d = nc.declare_dram_parameter("wq", [D, DH], PROJ_DT, isOutput=False)
    wk_d = nc.declare_dram_parameter("wk", [D, DH], PROJ_DT, isOutput=False)
    wv_d = nc.declare_dram_parameter("wv", [D, DH], PROJ_DT, isOutput=False)
    wo_d = nc.declare_dram_parameter("wo", [DH, D], VO_DT, isOutput=False)
    y_d = nc.declare_dram_parameter("y", [S, D], f32, isOutput=True)

    Exp = mybir.ActivationFunctionType.Exp

    with tile.TileContext(nc) as tc:
        with (
            tc.tile_pool(name="wpool", bufs=1) as wpool,
            tc.tile_pool(name="xpool", bufs=1) as xpool,
            tc.tile_pool(name="qkpool", bufs=1) as qkpool,
            tc.tile_pool(name="vpool", bufs=1) as vpool,
            tc.tile_pool(name="vnpool", bufs=1) as vnpool,
            tc.tile_pool(name="epool", bufs=29) as epool,
            tc.tile_pool(name="spool", bufs=2) as spool,
            tc.tile_pool(name="ypool", bufs=4) as ypool,
            tc.tile_pool(name="ps_proj", bufs=2, space="PSUM") as ps_proj,
            tc.tile_pool(name="ps_score", bufs=2, space="PSUM") as ps_score,
            tc.tile_pool(name="ps_av", bufs=2, space="PSUM") as ps_av_pool,
        ):
            # ---- load inputs to SBUF ----
            # Coarse 3D DMAs in compute-priority order: qk_proj(0) needs x
            # plus only the m0 columns of wq/wk, so those land first and the
            # first projection chain starts ~15us earlier than with uniform
            # per-tile loads.
            xt_big = []
            for j in range(4):
                tl = xpool.tile([128, 2 * S], PROJ_DT, tag=f"xtb{j}", name=f"xtb{j}")
                xt_big.append(tl)
            wbig = {}
            for nm in ("wq", "wk", "wv"):
                wbig[nm] = [
                    wpool.tile([128, 4 * DH], PROJ_DT, tag=f"{nm}b{j}", name=f"{nm}b{j}")
                    for j in range(2)
                ]
            wo_big = wpool.tile([128, 4 * D], VO_DT, tag="wob", name="wob")

            def dma_x(j):
                src = xt_d[j * 256:(j + 1) * 256, :].rearrange(
                    "(g p) s -> p g s", p=128)
                nc.sync.dma_start(
                    out=xt_big[j].rearrange("p (g s) -> p g s", g=2), in_=src)

            def dma_w(nm, dram, j, cols=slice(0, DH)):
                src = dram[j * 512:(j + 1) * 512, cols].rearrange(
                    "(g p) c -> p g c", p=128)
                nc.sync.dma_start(
                    out=wbig[nm][j].rearrange(
                        "p (g c) -> p g c", g=4)[:, :, cols], in_=src)

            m0, mrest = slice(0, 128), slice(128, DH)
            dma_x(0)
            dma_w("wq", wq_d, 0, m0)
            dma_w("wk", wk_d, 0, m0)
            dma_x(1)
            dma_w("wq", wq_d, 1, m0)
            dma_w("wk", wk_d, 1, m0)
            dma_x(2)
            dma_x(3)
            dma_w("wv", wv_d, 0)
            dma_w("wv", wv_d, 1)
            dma_w("wq", wq_d, 0, mrest)
            dma_w("wk", wk_d, 0, mrest)
            dma_w("wq", wq_d, 1, mrest)
            dma_w("wk", wk_d, 1, mrest)
            nc.sync.dma_start(
                out=wo_big.rearrange("p (g c) -> p g c", g=4),
                in_=wo_d.rearrange("(g p) c -> p g c", p=128),
            )

            xt = [xt_big[t // 2][:, (t % 2) * S:(t % 2 + 1) * S] for t in range(DT_IN)]
            wq = [wbig["wq"][t // 4][:, (t % 4) * DH:(t % 4 + 1) * DH] for t in range(DT_IN)]
            wk = [wbig["wk"][t // 4][:, (t % 4) * DH:(t % 4 + 1) * DH] for t in range(DT_IN)]
            wv = [wbig["wv"][t // 4][:, (t % 4) * DH:(t % 4 + 1) * DH] for t in range(DT_IN)]
            wo = [wo_big[:, t * D:(t + 1) * D] for t in range(MT)]

            # ---- compute body (repeatable for timing runs) ----
            for _rep in range(repeat):
                _emit_body(nc, tc, xt, wq, wk, wv, wo, y_d,
                           vpool, vnpool, qkpool, epool, spool, ypool,
                           ps_proj, ps_score, ps_av_pool)

    nc.compile()
    return nc


def _emit_body(nc, tc, xt, wq, wk, wv, wo, y_d,
               vpool, vnpool, qkpool, epool, spool, ypool,
               ps_proj, ps_score, ps_av_pool):
    f32 = mybir.dt.float32
    Exp = mybir.ActivationFunctionType.Exp

    v_sb = []
    for st in range(ST):
        v_sb.append(vpool.tile([128, HH * 66], E_DT, tag=f"v{st}", name=f"v{st}"))

    def emit_v_chain(st):
        # v_sb[st] = [128 seq, 8 heads x (64 v | 1 one | pad)]
        vt = v_sb[st]
        p = ps_proj.tile([128, DH], f32, tag="pp", name="pvp")
        for t in range(DT_IN):
            nc.tensor.matmul(
                p, xt[t][:, st * 128:(st + 1) * 128], wv[t],
                start=(t == 0), stop=(t == DT_IN - 1),
            )
        nc.vector.tensor_copy(
            vt.rearrange("p (h w) -> p h w", w=66)[:, :, 0:64],
            p.rearrange("p (h w) -> p h w", w=64),
        )
        nc.vector.memset(vt.rearrange("p (h w) -> p h w", w=66)[:, :, 64:65], 1.0)

    qk = {}

    def emit_qk_alloc(m):
        qT = qkpool.tile([128, S], QK_DT, tag=f"qT{m}", name=f"qT{m}")
        kT = qkpool.tile([128, S], QK_DT, tag=f"kT{m}", name=f"kT{m}")
        qk[m] = (qT, kT)

    def emit_qk_chains(m, cs):
        # chain index c in 0..7: c%2 selects q/k, c//2 selects the q-chunk
        qT, kT = qk[m]
        for c in cs:
            dst = qT if c % 2 == 0 else kT
            w = wq if c % 2 == 0 else wk
            nck = c // 2
            p = ps_proj.tile([128, 512], f32, tag="pp", name="pqk")
            for t in range(DT_IN):
                nc.tensor.matmul(
                    p,
                    w[t][:, m * 128:(m + 1) * 128],
                    xt[t][:, nck * 512:(nck + 1) * 512],
                    start=(t == 0), stop=(t == DT_IN - 1),
                )
            nc.vector.tensor_copy(dst[:, nck * 512:(nck + 1) * 512], p)

    def emit_qk_proj(m):
        # scores(0,0) needs all of kT but only the first q-chunk of qT:
        # emit k0+q0 first (first exp possible after 2 chains), then the
        # remaining k chains; q1..q3 are deferred into later windows.
        emit_qk_alloc(m)
        emit_qk_chains(m, [1, 0, 3, 5, 7])

    valsn = []
    for m in range(MT):
        vn = vnpool.tile([128, S], VO_DT, tag=f"vn{m}")
        valsn.append(vn)

    def emit_scores(m, qc, per_kt=None):
        qT, kT = qk[m]
        qsl = slice(qc * 512, (qc + 1) * 512)
        E = []
        for kt in range(KT):
            if per_kt is not None:
                per_kt(kt)
            ksl = slice(kt * 128, (kt + 1) * 128)
            ps = ps_score.tile([128, 1024], f32, tag="score")
            if "scores" not in SKIP:
                nc.tensor.matmul(
                    ps[:, 0:512], kT[0:64, ksl], qT[0:64, qsl],
                    start=True, stop=True,
                )
                nc.tensor.matmul(
                    ps[:, 512:1024], kT[64:128, ksl], qT[64:128, qsl],
                    start=True, stop=True,
                )
            else:
                nc.tensor.matmul(
                    ps[0:8, 0:8], kT[0:64, kt * 128:kt * 128 + 8], qT[0:64, qsl][:, 0:8],
                    start=True, stop=True,
                )
                nc.tensor.matmul(
                    ps[0:8, 512:520], kT[64:128, kt * 128:kt * 128 + 8], qT[64:128, qsl][:, 0:8],
                    start=True, stop=True,
                )
            e = epool.tile([128, 1024], E_DT, tag="e")
            if "exp" not in SKIP:
                nc.scalar.activation(e, ps, Exp)
            else:
                nc.scalar.activation(e[0:8, 0:8], ps[0:8, 0:8], Exp)
            E.append(e)
        return E

    def emit_av(m, qc, E):
        # The two head-par chains run kt-interleaved on the two AV banks
        # (accumulating matmuls never hit the same bank twice in a row),
        # and normalization reads each PSUM bank directly: no cross-bank
        # merge adds on DVE.
        qsl = slice(qc * 512, (qc + 1) * 512)
        pav = [ps_av_pool.tile([65, 512], f32, tag="av", name="pav")
               for _ in range(2)]
        for kt in range(KT):
            for par in range(2):
                h = 2 * m + par
                nc.tensor.matmul(
                    pav[par],
                    v_sb[kt][:, h * 66:h * 66 + 65],
                    E[kt][:, par * 512:(par + 1) * 512],
                    start=(kt == 0), stop=(kt == KT - 1),
                )
        for par in range(2):
            psl = slice(par * 64, (par + 1) * 64)
            srow0 = spool.tile([1, 512], f32, tag="srow0", name="srow0")
            # reciprocal with partition shift 64 -> 0, straight from PSUM
            nc.vector.reciprocal(srow0, pav[par][64:65, :])
            bc = spool.tile([128, 512], f32, tag="bc", name="bc")
            nc.gpsimd.partition_broadcast(bc, srow0)
            if par == 0:
                nc.vector.tensor_mul(
                    valsn[m][0:64, qsl], pav[0][0:64, :], bc[0:64, :])
            else:
                avs = spool.tile([128, 512], f32, tag="avs", name="avs")
                nc.vector.tensor_copy(avs[psl, :], pav[1][0:64, :])
                nc.vector.tensor_mul(valsn[m][psl, qsl], avs[psl, :], bc[psl, :])

    def emit_outproj_block(qb):
        _emit_outproj_block_impl(nc, qb, valsn, wo, y_d, ps_proj, ypool)

    # Emission order tuned for overlap: get the first scores to ScalarE as
    # early as possible (exp is the bottleneck engine), slot the v
    # projection behind the first score batch, and interleave the next
    # pair's q/k projection into the middle of the current pair's attention.
    emit_qk_proj(0)
    spans = [[0, 1, 2], [3, 4, 5], [6, 7]]
    for m in range(MT):
        if m + 1 < MT:
            emit_qk_alloc(m + 1)
        for qc in range(4):
            if m == 0 and qc == 0:
                # one v-projection chain between successive score units so
                # the first AV can start as soon as possible after (0,0)
                E = emit_scores(m, qc, per_kt=lambda kt: emit_v_chain(kt))
            else:
                E = emit_scores(m, qc)
            if m == 0 and qc < 3:
                # deferred q-chunk projections for m=0 (chain 2=q1,4=q2,6=q3)
                emit_qk_chains(0, [2 * (qc + 1)])
            if qc < 3 and m + 1 < MT:
                emit_qk_chains(m + 1, spans[qc])
            emit_av(m, qc, E)
            if m == MT - 1:
                emit_outproj_block(qc)


def _emit_outproj_block_impl(nc, qb, valsn, wo, y_d, ps_proj, ypool):
    f32 = mybir.dt.float32
    for st in range(4 * qb, 4 * qb + 4):
        ssl = slice(st * 128, (st + 1) * 128)
        for oc in range(2):
            osl = slice(oc * 512, (oc + 1) * 512)
            p = ps_proj.tile([128, 512], f32, tag="pp", name="pop")
            for t in range(MT):
                nc.tensor.matmul(
                    p, valsn[t][:, ssl], wo[t][:, osl],
                    start=(t == 0), stop=(t == MT - 1),
                )
            ys = ypool.tile([128, 512], f32, tag="y", name="ys")
            nc.vector.tensor_copy(ys, p)
            nc.sync.dma_start(out=y_d[ssl, osl], in_=ys)


def _prep_core_inputs(x, Wq, bq, Wk, bk, Wv, bv, Wo):
    """Host-side shard prep. Returns list of per-core input dicts."""
    pnp = _NP[PROJ_DT]
    vnp = _NP[VO_DT]
    wq_s = (Wq * SCALE).astype(pnp)
    wk_s = Wk.astype(pnp)
    wv_s = Wv.astype(pnp)
    in_maps = []
    for c in range(NCORES):
        b = c // 2
        hh = c % 2
        cols = slice(hh * DH, (hh + 1) * DH)
        in_maps.append({
            "xt": np.ascontiguousarray(x[b].T).astype(pnp),
            "wq": np.ascontiguousarray(wq_s[:, cols]),
            "wk": np.ascontiguousarray(wk_s[:, cols]),
            "wv": np.ascontiguousarray(wv_s[:, cols]),
            "wo": np.ascontiguousarray(Wo[cols, :]).astype(vnp),
        })
    return in_maps


def _numpy_mha(x, Wq, bq, Wk, bk, Wv, bv, Wo, bo):
    y = np.empty((B, S, D), dtype=np.float32)
    for b in range(B):
        q = (x[b] @ Wq + bq).reshape(S, H, Hd).transpose(1, 0, 2)
        k = (x[b] @ Wk + bk).reshape(S, H, Hd).transpose(1, 0, 2)
        v = (x[b] @ Wv + bv).reshape(S, H, Hd).transpose(1, 0, 2)
        vals = np.empty((H, S, Hd), dtype=np.float32)
        for h in range(H):
            lg = (q[h] @ k[h].T) * SCALE
            lg -= lg.max(axis=-1, keepdims=True)
            e = np.exp(lg)
            vals[h] = (e @ v[h]) / e.sum(axis=-1, keepdims=True)
        y[b] = vals.transpose(1, 0, 2).reshape(S, D) @ Wo + bo
    return y


def kernel(x, Wq, bq, Wk, bk, Wv, bv, Wo, bo):
    x = np.asarray(x, dtype=np.float32)
    Wq = np.asarray(Wq, dtype=np.float32)
    Wk = np.asarray(Wk, dtype=np.float32)
    Wv = np.asarray(Wv, dtype=np.float32)
    Wo = np.asarray(Wo, dtype=np.float32)
    bq = np.asarray(bq, dtype=np.float32)
    bk = np.asarray(bk, dtype=np.float32)
    bv = np.asarray(bv, dtype=np.float32)
    bo = np.asarray(bo, dtype=np.float32)
    if max(np.abs(bq).max(), np.abs(bk).max(), np.abs(bv).max()) != 0:
        # The reference always uses zero q/k/v biases; keep a host fallback
        # for generality rather than failing.
        return _numpy_mha(x, Wq, bq, Wk, bk, Wv, bv, Wo, bo)

    if "nc" not in _CACHE:
        _CACHE["nc"] = _build_program()
    nc = _CACHE["nc"]

    in_maps = _prep_core_inputs(x, Wq, bq, Wk, bk, Wv, bv, Wo)
    res = run_bass_kernel_spmd(nc, in_maps, list(range(NCORES)))

    y = np.empty((B, S, D), dtype=np.float32)
    for b in range(B):
        y[b] = res.results[2 * b]["y"] + res.results[2 * b + 1]["y"]
    y += bo[None, None, :]
    return y



# revision 19
# speedup vs baseline: 1.0192x; 1.0192x over previous
"""Multi-head self-attention (B=4, S=2048, D=1024, H=16) on 8 Trainium2 cores.

Sharding: core c handles batch c//2 and head-half c%2 (8 heads = 512 dims).
Each core computes q/k/v projections for its heads, attention, and a partial
output projection (contraction over its 512 head-dims). Host sums the two
partials per batch and adds bo.

Schedule (HW-A/B-tested against the previous versions):
- qk_proj(0) emits k-chains + the first q-chunk chain first; scores (and
  ScalarE exp) start after 2 chains instead of 8. q1..q3 stream later.
- The two head-par AV chains run kt-interleaved on the two AV PSUM banks
  (no same-bank back-to-back accumulation) and normalization reads each
  bank directly: no cross-bank merge adds on DVE (~40us less DVE busy).
Earlier findings kept:
- Coarse 3D prologue DMAs in compute-priority order (x + m0 columns of
  wq/wk first): the first projection chain starts ~9us earlier.
- q/k projection chains for head-pair m+1 spread 3/3/2 over the qc=0..2
  windows of m instead of one lump at qc==2 (no PE burst at m boundaries).
- v-projection chains interleaved one-per-score-unit inside (0,0) so the
  first AV chain is unblocked as early as possible; epool 25 bufs of
  backlog keep ScalarE fed across that window.
- m=3 AV drains immediately followed by that q-block's output projection,
  overlapping the output tail with the last attention windows.

Self-contained: hardcodes all shapes; only needs the environment-provided
concourse libraries.
"""
import os
import sys

for _p in ("/opt/trn_rl_repo", "/root/.axon_site/_ro/trn_rl_repo"):
    if os.path.isdir(_p) and _p not in sys.path:
        sys.path.insert(0, _p)

import numpy as np
import ml_dtypes

import concourse.bass as bass  # noqa: F401
from concourse import bacc
import concourse.mybir as mybir
import concourse.tile as tile
from concourse.bass_utils import run_bass_kernel_spmd

# problem constants
B, S, D, H, Hd = 4, 2048, 1024, 16, 64
NCORES = 8
HH = H // 2            # heads per core
DH = HH * Hd           # head dims per core = 512
SCALE = 1.0 / np.sqrt(Hd)

ST = S // 128          # 16 seq tiles
QC = S // 512          # 4 q chunks
KT = ST                # 16 k tiles
DT_IN = D // 128       # 8 input-dim tiles
MT = DH // 128         # 4 head-pair tiles

f32 = mybir.dt.float32
bf16 = mybir.dt.bfloat16
np_bf16 = ml_dtypes.bfloat16

# matmul input dtypes per stage (fp32 accumulation always).
# float16: 1 cy/row on PE like bf16, 8x finer mantissa. exp(logit) must stay
# below 65504 — max |logit| for randn-style inputs is ~9 (exp ~6.6e3), safe.
PROJ_DT = mybir.dt.float16    # xT / wq / wk / wv
QK_DT = mybir.dt.float16      # qT / kT (scores matmul operands)
E_DT = mybir.dt.float16       # exp outputs and v_aug (AV matmul operands)
VO_DT = mybir.dt.float16      # valsn / wo (output projection operands)
_NP = {
    mybir.dt.bfloat16: np_bf16,
    mybir.dt.float16: np.float16,
    mybir.dt.float32r: np.float32,
    mybir.dt.float32: np.float32,
}

_CACHE = {}

# timing-ablation knob: subset of {"scores", "exp", "av", "outproj", "norm"}
# suppresses those instruction groups (breaks semantics, timing only)
SKIP = set()


def _build_program(repeat=1):
    nc = bacc.Bacc("TRN2", target_bir_lowering=False, debug=False)

    xt_d = nc.declare_dram_parameter("xt", [D, S], PROJ_DT, isOutput=False)
    wq_d = nc.declare_dram_parameter("wq", [D, DH], PROJ_DT, isOutput=False)
    wk_d = nc.declare_dram_parameter("wk", [D, DH], PROJ_DT, isOutput=False)
    wv_d = nc.declare_dram_parameter("wv", [D, DH], PROJ_DT, isOutput=False)
    wo_d = nc.declare_dram_parameter("wo", [DH, D], VO_DT, isOutput=False)
    y_d = nc.declare_dram_parameter("y", [S, D], f32, isOutput=True)

    Exp = mybir.ActivationFunctionType.Exp

    with tile.TileContext(nc) as tc:
        with (
            tc.tile_pool(name="wpool", bufs=1) as wpool,
            tc.tile_pool(name="xpool", bufs=1) as xpool,
            tc.tile_pool(name="qkpool", bufs=1) as qkpool,
            tc.tile_pool(name="vpool", bufs=1) as vpool,
            tc.tile_pool(name="vnpool", bufs=1) as vnpool,
            tc.tile_pool(name="epool", bufs=29) as epool,
            tc.tile_pool(name="spool", bufs=2) as spool,
            tc.tile_pool(name="ypool", bufs=2) as ypool,
            tc.tile_pool(name="ps_proj", bufs=2, space="PSUM") as ps_proj,
            tc.tile_pool(name="ps_score", bufs=2, space="PSUM") as ps_score,
            tc.tile_pool(name="ps_av", bufs=2, space="PSUM") as ps_av_pool,
        ):
            # ---- load inputs to SBUF ----
            # Coarse 3D DMAs in compute-priority order: qk_proj(0) needs x
            # plus only the m0 columns of wq/wk, so those land first and the
            # first projection chain starts ~15us earlier than with uniform
            # per-tile loads.
            xt_big = []
            for j in range(4):
                tl = xpool.tile([128, 2 * S], PROJ_DT, tag=f"xtb{j}", name=f"xtb{j}")
                xt_big.append(tl)
            wbig = {}
            for nm in ("wq", "wk", "wv"):
                wbig[nm] = [
                    wpool.tile([128, 4 * DH], PROJ_DT, tag=f"{nm}b{j}", name=f"{nm}b{j}")
                    for j in range(2)
                ]
            wo_big = wpool.tile([128, 4 * D], VO_DT, tag="wob", name="wob")

            def dma_x(j):
                src = xt_d[j * 256:(j + 1) * 256, :].rearrange(
                    "(g p) s -> p g s", p=128)
                nc.sync.dma_start(
                    out=xt_big[j].rearrange("p (g s) -> p g s", g=2), in_=src)

            def dma_w(nm, dram, j, cols=slice(0, DH)):
                src = dram[j * 512:(j + 1) * 512, cols].rearrange(
                    "(g p) c -> p g c", p=128)
                nc.sync.dma_start(
                    out=wbig[nm][j].rearrange(
                        "p (g c) -> p g c", g=4)[:, :, cols], in_=src)

            m0, mrest = slice(0, 128), slice(128, DH)
            dma_x(0)
            dma_w("wq", wq_d, 0, m0)
            dma_w("wk", wk_d, 0, m0)
            dma_x(1)
            dma_w("wq", wq_d, 1, m0)
            dma_w("wk", wk_d, 1, m0)
            dma_x(2)
            dma_x(3)
            dma_w("wv", wv_d, 0)
            dma_w("wv", wv_d, 1)
            dma_w("wq", wq_d, 0, mrest)
            dma_w("wk", wk_d, 0, mrest)
            dma_w("wq", wq_d, 1, mrest)
            dma_w("wk", wk_d, 1, mrest)
            nc.sync.dma_start(
                out=wo_big.rearrange("p (g c) -> p g c", g=4),
                in_=wo_d.rearrange("(g p) c -> p g c", p=128),
            )

            xt = [xt_big[t // 2][:, (t % 2) * S:(t % 2 + 1) * S] for t in range(DT_IN)]
            wq = [wbig["wq"][t // 4][:, (t % 4) * DH:(t % 4 + 1) * DH] for t in range(DT_IN)]
            wk = [wbig["wk"][t // 4][:, (t % 4) * DH:(t % 4 + 1) * DH] for t in range(DT_IN)]
            wv = [wbig["wv"][t // 4][:, (t % 4) * DH:(t % 4 + 1) * DH] for t in range(DT_IN)]
            wo = [wo_big[:, t * D:(t + 1) * D] for t in range(MT)]

            # ---- compute body (repeatable for timing runs) ----
            for _rep in range(repeat):
                _emit_body(nc, tc, xt, wq, wk, wv, wo, y_d,
                           vpool, vnpool, qkpool, epool, spool, ypool,
                           ps_proj, ps_score, ps_av_pool)

    nc.compile()
    return nc


def _emit_body(nc, tc, xt, wq, wk, wv, wo, y_d,
               vpool, vnpool, qkpool, epool, spool, ypool,
               ps_proj, ps_score, ps_av_pool):
    f32 = mybir.dt.float32
    Exp = mybir.ActivationFunctionType.Exp

    v_sb = []
    for st in range(ST):
        v_sb.append(vpool.tile([128, HH * 66], E_DT, tag=f"v{st}", name=f"v{st}"))

    def emit_v_chain(st):
        # v_sb[st] = [128 seq, 8 heads x (64 v | 1 one | pad)]
        vt = v_sb[st]
        p = ps_proj.tile([128, DH], f32, tag="pp", name="pvp")
        for t in range(DT_IN):
            nc.tensor.matmul(
                p, xt[t][:, st * 128:(st + 1) * 128], wv[t],
                start=(t == 0), stop=(t == DT_IN - 1),
            )
        nc.vector.tensor_copy(
            vt.rearrange("p (h w) -> p h w", w=66)[:, :, 0:64],
            p.rearrange("p (h w) -> p h w", w=64),
        )
        nc.vector.memset(vt.rearrange("p (h w) -> p h w", w=66)[:, :, 64:65], 1.0)

    qk = {}

    def emit_qk_alloc(m):
        qT = qkpool.tile([128, S], QK_DT, tag=f"qT{m}", name=f"qT{m}")
        kT = qkpool.tile([128, S], QK_DT, tag=f"kT{m}", name=f"kT{m}")
        qk[m] = (qT, kT)

    def emit_qk_chains(m, cs):
        # chain index c in 0..7: c%2 selects q/k, c//2 selects the q-chunk
        qT, kT = qk[m]
        for c in cs:
            dst = qT if c % 2 == 0 else kT
            w = wq if c % 2 == 0 else wk
            nck = c // 2
            p = ps_proj.tile([128, 512], f32, tag="pp", name="pqk")
            for t in range(DT_IN):
                nc.tensor.matmul(
                    p,
                    w[t][:, m * 128:(m + 1) * 128],
                    xt[t][:, nck * 512:(nck + 1) * 512],
                    start=(t == 0), stop=(t == DT_IN - 1),
                )
            nc.vector.tensor_copy(dst[:, nck * 512:(nck + 1) * 512], p)

    def emit_qk_proj(m):
        # scores(0,0) needs all of kT but only the first q-chunk of qT:
        # emit k0+q0 first (first exp possible after 2 chains), then the
        # remaining k chains; q1..q3 are deferred into later windows.
        emit_qk_alloc(m)
        emit_qk_chains(m, [1, 0, 3, 5, 7])

    valsn = []
    for m in range(MT):
        vn = vnpool.tile([128, S], VO_DT, tag=f"vn{m}")
        valsn.append(vn)

    def emit_scores(m, qc, per_kt=None):
        qT, kT = qk[m]
        qsl = slice(qc * 512, (qc + 1) * 512)
        E = []
        for kt in range(KT):
            if per_kt is not None:
                per_kt(kt)
            ksl = slice(kt * 128, (kt + 1) * 128)
            ps = ps_score.tile([128, 1024], f32, tag="score")
            if "scores" not in SKIP:
                nc.tensor.matmul(
                    ps[:, 0:512], kT[0:64, ksl], qT[0:64, qsl],
                    start=True, stop=True,
                )
                nc.tensor.matmul(
                    ps[:, 512:1024], kT[64:128, ksl], qT[64:128, qsl],
                    start=True, stop=True,
                )
            else:
                nc.tensor.matmul(
                    ps[0:8, 0:8], kT[0:64, kt * 128:kt * 128 + 8], qT[0:64, qsl][:, 0:8],
                    start=True, stop=True,
                )
                nc.tensor.matmul(
                    ps[0:8, 512:520], kT[64:128, kt * 128:kt * 128 + 8], qT[64:128, qsl][:, 0:8],
                    start=True, stop=True,
                )
            e = epool.tile([128, 1024], E_DT, tag="e")
            if "exp" not in SKIP:
                nc.scalar.activation(e, ps, Exp)
            else:
                nc.scalar.activation(e[0:8, 0:8], ps[0:8, 0:8], Exp)
            E.append(e)
        return E

    def emit_av(m, qc, E):
        # The two head-par chains run kt-interleaved on the two AV banks
        # (accumulating matmuls never hit the same bank twice in a row),
        # and normalization reads each PSUM bank directly: no cross-bank
        # merge adds on DVE.
        qsl = slice(qc * 512, (qc + 1) * 512)
        pav = [ps_av_pool.tile([65, 512], f32, tag="av", name="pav")
               for _ in range(2)]
        for kt in range(KT):
            for par in range(2):
                h = 2 * m + par
                nc.tensor.matmul(
                    pav[par],
                    v_sb[kt][:, h * 66:h * 66 + 65],
                    E[kt][:, par * 512:(par + 1) * 512],
                    start=(kt == 0), stop=(kt == KT - 1),
                )
        for par in range(2):
            psl = slice(par * 64, (par + 1) * 64)
            # single fast copy releases the PSUM bank for the next AV pair;
            # the rest of the normalization runs SBUF-side
            tmp = spool.tile([65, 512], f32, tag="tmp", name="tmp")
            nc.vector.tensor_copy(tmp, pav[par])
            srow0 = spool.tile([1, 512], f32, tag="srow0", name="srow0")
            nc.vector.reciprocal(srow0, tmp[64:65, :])
            bc = spool.tile([128, 512], f32, tag="bc", name="bc")
            nc.gpsimd.partition_broadcast(bc, srow0)
            if par == 0:
                nc.vector.tensor_mul(
                    valsn[m][0:64, qsl], tmp[0:64, :], bc[0:64, :])
            else:
                avs = spool.tile([128, 512], f32, tag="avs", name="avs")
                nc.vector.tensor_copy(avs[psl, :], tmp[0:64, :])
                nc.vector.tensor_mul(valsn[m][psl, qsl], avs[psl, :], bc[psl, :])

    def emit_outproj_block(qb):
        _emit_outproj_block_impl(nc, qb, valsn, wo, y_d, ps_proj, ypool)

    # Emission order tuned for overlap: get the first scores to ScalarE as
    # early as possible (exp is the bottleneck engine), slot the v
    # projection behind the first score batch, and interleave the next
    # pair's q/k projection into the middle of the current pair's attention.
    emit_qk_proj(0)
    spans = [[0, 1, 2], [3, 4, 5], [6, 7]]
    for m in range(MT):
        if m + 1 < MT:
            emit_qk_alloc(m + 1)
        for qc in range(4):
            if m == 0 and qc == 0:
                # one v-projection chain between successive score units so
                # the first AV can start as soon as possible after (0,0)
                E = emit_scores(m, qc, per_kt=lambda kt: emit_v_chain(kt))
            else:
                E = emit_scores(m, qc)
            if m == 0 and qc < 3:
                # deferred q-chunk projections for m=0 (chain 2=q1,4=q2,6=q3)
                emit_qk_chains(0, [2 * (qc + 1)])
            if qc < 3 and m + 1 < MT:
                emit_qk_chains(m + 1, spans[qc])
            emit_av(m, qc, E)
            if m == MT - 1:
                emit_outproj_block(qc)


def _emit_outproj_block_impl(nc, qb, valsn, wo, y_d, ps_proj, ypool):
    f32 = mybir.dt.float32
    for st in range(4 * qb, 4 * qb + 4):
        ssl = slice(st * 128, (st + 1) * 128)
        for oc in range(2):
            osl = slice(oc * 512, (oc + 1) * 512)
            p = ps_proj.tile([128, 512], f32, tag="pp", name="pop")
            for t in range(MT):
                nc.tensor.matmul(
                    p, valsn[t][:, ssl], wo[t][:, osl],
                    start=(t == 0), stop=(t == MT - 1),
                )
            ys = ypool.tile([128, 512], f32, tag="y", name="ys")
            nc.vector.tensor_copy(ys, p)
            nc.sync.dma_start(out=y_d[ssl, osl], in_=ys)


def _prep_core_inputs(x, Wq, bq, Wk, bk, Wv, bv, Wo):
    """Host-side shard prep. Returns list of per-core input dicts."""
    pnp = _NP[PROJ_DT]
    vnp = _NP[VO_DT]
    wq_s = (Wq * SCALE).astype(pnp)
    wk_s = Wk.astype(pnp)
    wv_s = Wv.astype(pnp)
    in_maps = []
    for c in range(NCORES):
        b = c // 2
        hh = c % 2
        cols = slice(hh * DH, (hh + 1) * DH)
        in_maps.append({
            "xt": np.ascontiguousarray(x[b].T).astype(pnp),
            "wq": np.ascontiguousarray(wq_s[:, cols]),
            "wk": np.ascontiguousarray(wk_s[:, cols]),
            "wv": np.ascontiguousarray(wv_s[:, cols]),
            "wo": np.ascontiguousarray(Wo[cols, :]).astype(vnp),
        })
    return in_maps


def _numpy_mha(x, Wq, bq, Wk, bk, Wv, bv, Wo, bo):
    y = np.empty((B, S, D), dtype=np.float32)
    for b in range(B):
        q = (x[b] @ Wq + bq).reshape(S, H, Hd).transpose(1, 0, 2)
        k = (x[b] @ Wk + bk).reshape(S, H, Hd).transpose(1, 0, 2)
        v = (x[b] @ Wv + bv).reshape(S, H, Hd).transpose(1, 0, 2)
        vals = np.empty((H, S, Hd), dtype=np.float32)
        for h in range(H):
            lg = (q[h] @ k[h].T) * SCALE
            lg -= lg.max(axis=-1, keepdims=True)
            e = np.exp(lg)
            vals[h] = (e @ v[h]) / e.sum(axis=-1, keepdims=True)
        y[b] = vals.transpose(1, 0, 2).reshape(S, D) @ Wo + bo
    return y


def kernel(x, Wq, bq, Wk, bk, Wv, bv, Wo, bo):
    x = np.asarray(x, dtype=np.float32)
    Wq = np.asarray(Wq, dtype=np.float32)
    Wk = np.asarray(Wk, dtype=np.float32)
    Wv = np.asarray(Wv, dtype=np.float32)
    Wo = np.asarray(Wo, dtype=np.float32)
    bq = np.asarray(bq, dtype=np.float32)
    bk = np.asarray(bk, dtype=np.float32)
    bv = np.asarray(bv, dtype=np.float32)
    bo = np.asarray(bo, dtype=np.float32)
    if max(np.abs(bq).max(), np.abs(bk).max(), np.abs(bv).max()) != 0:
        # The reference always uses zero q/k/v biases; keep a host fallback
        # for generality rather than failing.
        return _numpy_mha(x, Wq, bq, Wk, bk, Wv, bv, Wo, bo)

    if "nc" not in _CACHE:
        _CACHE["nc"] = _build_program()
    nc = _CACHE["nc"]

    in_maps = _prep_core_inputs(x, Wq, bq, Wk, bk, Wv, bv, Wo)
    res = run_bass_kernel_spmd(nc, in_maps, list(range(NCORES)))

    y = np.empty((B, S, D), dtype=np.float32)
    for b in range(B):
        y[b] = res.results[2 * b]["y"] + res.results[2 * b + 1]["y"]
    y += bo[None, None, :]
    return y



# revision 21
# speedup vs baseline: 1.1009x; 1.0801x over previous
"""Multi-head self-attention (B=4, S=2048, D=1024, H=16) on 8 Trainium2 cores.

Sharding: core c handles batch c//2 and head-half c%2 (8 heads = 512 dims).
Each core computes q/k/v projections for its heads, attention, and a partial
output projection (contraction over its 512 head-dims). Host sums the two
partials per batch and adds bo.

Schedule (HW-A/B-tested against the previous versions):
- qk_proj(0) emits k-chains + the first q-chunk chain first; scores (and
  ScalarE exp) start after 2 chains instead of 8. q1..q3 stream later.
- The two head-par AV chains run kt-interleaved on the two AV PSUM banks
  (no same-bank back-to-back accumulation); a single fast DVE copy drains
  each bank (released ~1.4us sooner for the next AV pair) and the
  normalization (reciprocal+broadcast+mul) runs SBUF-side. No cross-bank
  merge adds.
Earlier findings kept:
- Coarse 3D prologue DMAs in compute-priority order (x + m0 columns of
  wq/wk first): the first projection chain starts ~9us earlier.
- q/k projection chains for head-pair m+1 spread 3/3/2 over the qc=0..2
  windows of m instead of one lump at qc==2 (no PE burst at m boundaries).
- v-projection chains interleaved one-per-score-unit inside (0,0) so the
  first AV chain is unblocked as early as possible; epool 25 bufs of
  backlog keep ScalarE fed across that window.
- m=3 AV drains immediately followed by that q-block's output projection,
  overlapping the output tail with the last attention windows.

Self-contained: hardcodes all shapes; only needs the environment-provided
concourse libraries.
"""
import os
import sys

for _p in ("/opt/trn_rl_repo", "/root/.axon_site/_ro/trn_rl_repo"):
    if os.path.isdir(_p) and _p not in sys.path:
        sys.path.insert(0, _p)

import numpy as np
import ml_dtypes

import concourse.bass as bass  # noqa: F401
from concourse import bacc
import concourse.mybir as mybir
import concourse.tile as tile
from concourse.bass_utils import run_bass_kernel_spmd

# problem constants
B, S, D, H, Hd = 4, 2048, 1024, 16, 64
NCORES = 8
HH = H // 2            # heads per core
DH = HH * Hd           # head dims per core = 512
SCALE = 1.0 / np.sqrt(Hd)

ST = S // 128          # 16 seq tiles
QC = S // 512          # 4 q chunks
KT = ST                # 16 k tiles
DT_IN = D // 128       # 8 input-dim tiles
MT = DH // 128         # 4 head-pair tiles

f32 = mybir.dt.float32
bf16 = mybir.dt.bfloat16
np_bf16 = ml_dtypes.bfloat16

# matmul input dtypes per stage (fp32 accumulation always).
# float16: 1 cy/row on PE like bf16, 8x finer mantissa. exp(logit) must stay
# below 65504 — max |logit| for randn-style inputs is ~9 (exp ~6.6e3), safe.
PROJ_DT = mybir.dt.float16    # xT / wq / wk / wv
QK_DT = mybir.dt.float16      # qT / kT (scores matmul operands)
E_DT = mybir.dt.float16       # exp outputs and v_aug (AV matmul operands)
VO_DT = mybir.dt.float16      # valsn / wo (output projection operands)
_NP = {
    mybir.dt.bfloat16: np_bf16,
    mybir.dt.float16: np.float16,
    mybir.dt.float32r: np.float32,
    mybir.dt.float32: np.float32,
}

_CACHE = {}

# timing-ablation knob: subset of {"scores", "exp", "av", "outproj", "norm"}
# suppresses those instruction groups (breaks semantics, timing only)
SKIP = set()


def _build_program(repeat=1):
    nc = bacc.Bacc("TRN2", target_bir_lowering=False, debug=False)

    xt_d = nc.declare_dram_parameter("xt", [D, S], PROJ_DT, isOutput=False)
    wq_d = nc.declare_dram_parameter("wq", [D, DH], PROJ_DT, isOutput=False)
    wk_d = nc.declare_dram_parameter("wk", [D, DH], PROJ_DT, isOutput=False)
    wv_d = nc.declare_dram_parameter("wv", [D, DH], PROJ_DT, isOutput=False)
    wo_d = nc.declare_dram_parameter("wo", [DH, D], VO_DT, isOutput=False)
    y_d = nc.declare_dram_parameter("y", [S, D], f32, isOutput=True)

    Exp = mybir.ActivationFunctionType.Exp

    with tile.TileContext(nc) as tc:
        with (
            tc.tile_pool(name="wpool", bufs=1) as wpool,
            tc.tile_pool(name="xpool", bufs=1) as xpool,
            tc.tile_pool(name="qkpool", bufs=1) as qkpool,
            tc.tile_pool(name="vpool", bufs=1) as vpool,
            tc.tile_pool(name="vnpool", bufs=1) as vnpool,
            tc.tile_pool(name="epool", bufs=29) as epool,
            tc.tile_pool(name="spool", bufs=2) as spool,
            tc.tile_pool(name="ypool", bufs=2) as ypool,
            tc.tile_pool(name="ps_proj", bufs=2, space="PSUM") as ps_proj,
            tc.tile_pool(name="ps_score", bufs=2, space="PSUM") as ps_score,
            tc.tile_pool(name="ps_av", bufs=2, space="PSUM") as ps_av_pool,
        ):
            # ---- load inputs to SBUF ----
            # Coarse 3D DMAs in compute-priority order: qk_proj(0) needs x
            # plus only the m0 columns of wq/wk, so those land first and the
            # first projection chain starts ~15us earlier than with uniform
            # per-tile loads.
            xt_big = []
            for j in range(4):
                tl = xpool.tile([128, 2 * S], PROJ_DT, tag=f"xtb{j}", name=f"xtb{j}")
                xt_big.append(tl)
            wbig = {}
            for nm in ("wq", "wk", "wv"):
                wbig[nm] = [
                    wpool.tile([128, 4 * DH], PROJ_DT, tag=f"{nm}b{j}", name=f"{nm}b{j}")
                    for j in range(2)
                ]
            wo_big = wpool.tile([128, 4 * D], VO_DT, tag="wob", name="wob")

            def dma_x(j):
                src = xt_d[j * 256:(j + 1) * 256, :].rearrange(
                    "(g p) s -> p g s", p=128)
                nc.sync.dma_start(
                    out=xt_big[j].rearrange("p (g s) -> p g s", g=2), in_=src)

            def dma_w(nm, dram, j, cols=slice(0, DH)):
                src = dram[j * 512:(j + 1) * 512, cols].rearrange(
                    "(g p) c -> p g c", p=128)
                nc.sync.dma_start(
                    out=wbig[nm][j].rearrange(
                        "p (g c) -> p g c", g=4)[:, :, cols], in_=src)

            m0, mrest = slice(0, 128), slice(128, DH)
            dma_x(0)
            # PE clock warm-up: ~3.4us of dummy matmuls (no data deps) during
            # the input-DMA wait trips the HAM activity window, so the real
            # projection chains start at full clock. Strictly shorter than
            # the minimum x-DMA time, so it never delays real work.
            warm_sb = xpool.tile([128, 640], PROJ_DT, tag="warm", name="warm")
            nc.vector.memset(warm_sb, 0.0)
            warm_ps = ps_score.tile([128, 1024], f32, tag="score", name="warmps")
            for _ in range(8):
                nc.tensor.matmul(
                    warm_ps[:, 0:512], warm_sb[:, 0:128], warm_sb[:, 128:640],
                    start=True, stop=True,
                )
            dma_w("wq", wq_d, 0, m0)
            dma_w("wk", wk_d, 0, m0)
            dma_x(1)
            dma_w("wq", wq_d, 1, m0)
            dma_w("wk", wk_d, 1, m0)
            dma_x(2)
            dma_x(3)
            dma_w("wv", wv_d, 0)
            dma_w("wv", wv_d, 1)
            dma_w("wq", wq_d, 0, mrest)
            dma_w("wk", wk_d, 0, mrest)
            dma_w("wq", wq_d, 1, mrest)
            dma_w("wk", wk_d, 1, mrest)
            nc.sync.dma_start(
                out=wo_big.rearrange("p (g c) -> p g c", g=4),
                in_=wo_d.rearrange("(g p) c -> p g c", p=128),
            )

            xt = [xt_big[t // 2][:, (t % 2) * S:(t % 2 + 1) * S] for t in range(DT_IN)]
            wq = [wbig["wq"][t // 4][:, (t % 4) * DH:(t % 4 + 1) * DH] for t in range(DT_IN)]
            wk = [wbig["wk"][t // 4][:, (t % 4) * DH:(t % 4 + 1) * DH] for t in range(DT_IN)]
            wv = [wbig["wv"][t // 4][:, (t % 4) * DH:(t % 4 + 1) * DH] for t in range(DT_IN)]
            wo = [wo_big[:, t * D:(t + 1) * D] for t in range(MT)]

            # ---- compute body (repeatable for timing runs) ----
            for _rep in range(repeat):
                _emit_body(nc, tc, xt, wq, wk, wv, wo, y_d,
                           vpool, vnpool, qkpool, epool, spool, ypool,
                           ps_proj, ps_score, ps_av_pool)

    nc.compile()
    return nc


def _emit_body(nc, tc, xt, wq, wk, wv, wo, y_d,
               vpool, vnpool, qkpool, epool, spool, ypool,
               ps_proj, ps_score, ps_av_pool):
    f32 = mybir.dt.float32
    Exp = mybir.ActivationFunctionType.Exp

    v_sb = []
    for st in range(ST):
        v_sb.append(vpool.tile([128, HH * 66], E_DT, tag=f"v{st}", name=f"v{st}"))

    def emit_v_chain(st):
        # v_sb[st] = [128 seq, 8 heads x (64 v | 1 one | pad)]
        vt = v_sb[st]
        p = ps_proj.tile([128, DH], f32, tag="pp", name="pvp")
        for t in range(DT_IN):
            nc.tensor.matmul(
                p, xt[t][:, st * 128:(st + 1) * 128], wv[t],
                start=(t == 0), stop=(t == DT_IN - 1),
            )
        nc.vector.tensor_copy(
            vt.rearrange("p (h w) -> p h w", w=66)[:, :, 0:64],
            p.rearrange("p (h w) -> p h w", w=64),
        )
        nc.vector.memset(vt.rearrange("p (h w) -> p h w", w=66)[:, :, 64:65], 1.0)

    qk = {}

    def emit_qk_alloc(m):
        qT = qkpool.tile([128, S], QK_DT, tag=f"qT{m}", name=f"qT{m}")
        kT = qkpool.tile([128, S], QK_DT, tag=f"kT{m}", name=f"kT{m}")
        qk[m] = (qT, kT)

    def emit_qk_chains(m, cs):
        # chain index c in 0..7: c%2 selects q/k, c//2 selects the q-chunk
        qT, kT = qk[m]
        for c in cs:
            dst = qT if c % 2 == 0 else kT
            w = wq if c % 2 == 0 else wk
            nck = c // 2
            p = ps_proj.tile([128, 512], f32, tag="pp", name="pqk")
            for t in range(DT_IN):
                nc.tensor.matmul(
                    p,
                    w[t][:, m * 128:(m + 1) * 128],
                    xt[t][:, nck * 512:(nck + 1) * 512],
                    start=(t == 0), stop=(t == DT_IN - 1),
                )
            nc.vector.tensor_copy(dst[:, nck * 512:(nck + 1) * 512], p)

    def emit_qk_proj(m):
        # scores(0,0) needs all of kT but only the first q-chunk of qT:
        # emit k0+q0 first (first exp possible after 2 chains), then the
        # remaining k chains; q1..q3 are deferred into later windows.
        emit_qk_alloc(m)
        emit_qk_chains(m, [1, 0, 3, 5, 7])

    valsn = []
    for m in range(MT):
        vn = vnpool.tile([128, S], VO_DT, tag=f"vn{m}")
        valsn.append(vn)

    def emit_scores(m, qc, per_kt=None):
        qT, kT = qk[m]
        qsl = slice(qc * 512, (qc + 1) * 512)
        E = []
        for kt in range(KT):
            if per_kt is not None:
                per_kt(kt)
            ksl = slice(kt * 128, (kt + 1) * 128)
            ps = ps_score.tile([128, 1024], f32, tag="score")
            if "scores" not in SKIP:
                nc.tensor.matmul(
                    ps[:, 0:512], kT[0:64, ksl], qT[0:64, qsl],
                    start=True, stop=True,
                )
                nc.tensor.matmul(
                    ps[:, 512:1024], kT[64:128, ksl], qT[64:128, qsl],
                    start=True, stop=True,
                )
            else:
                nc.tensor.matmul(
                    ps[0:8, 0:8], kT[0:64, kt * 128:kt * 128 + 8], qT[0:64, qsl][:, 0:8],
                    start=True, stop=True,
                )
                nc.tensor.matmul(
                    ps[0:8, 512:520], kT[64:128, kt * 128:kt * 128 + 8], qT[64:128, qsl][:, 0:8],
                    start=True, stop=True,
                )
            e = epool.tile([128, 1024], E_DT, tag="e")
            if "exp" not in SKIP:
                nc.scalar.activation(e, ps, Exp)
            else:
                nc.scalar.activation(e[0:8, 0:8], ps[0:8, 0:8], Exp)
            E.append(e)
        return E

    def emit_av(m, qc, E):
        # The two head-par chains run kt-interleaved on the two AV banks
        # (accumulating matmuls never hit the same bank twice in a row),
        # and normalization reads each PSUM bank directly: no cross-bank
        # merge adds on DVE.
        qsl = slice(qc * 512, (qc + 1) * 512)
        pav = [ps_av_pool.tile([65, 512], f32, tag="av", name="pav")
               for _ in range(2)]
        for kt in range(KT):
            for par in range(2):
                h = 2 * m + par
                nc.tensor.matmul(
                    pav[par],
                    v_sb[kt][:, h * 66:h * 66 + 65],
                    E[kt][:, par * 512:(par + 1) * 512],
                    start=(kt == 0), stop=(kt == KT - 1),
                )
        for par in range(2):
            psl = slice(par * 64, (par + 1) * 64)
            # single fast copy releases the PSUM bank for the next AV pair;
            # the rest of the normalization runs SBUF-side
            tmp = spool.tile([65, 512], f32, tag="tmp", name="tmp")
            nc.vector.tensor_copy(tmp, pav[par])
            srow0 = spool.tile([1, 512], f32, tag="srow0", name="srow0")
            nc.vector.reciprocal(srow0, tmp[64:65, :])
            bc = spool.tile([128, 512], f32, tag="bc", name="bc")
            nc.gpsimd.partition_broadcast(bc, srow0)
            if par == 0:
                nc.vector.tensor_mul(
                    valsn[m][0:64, qsl], tmp[0:64, :], bc[0:64, :])
            else:
                avs = spool.tile([128, 512], f32, tag="avs", name="avs")
                nc.vector.tensor_copy(avs[psl, :], tmp[0:64, :])
                nc.vector.tensor_mul(valsn[m][psl, qsl], avs[psl, :], bc[psl, :])

    def emit_outproj_block(qb):
        _emit_outproj_block_impl(nc, qb, valsn, wo, y_d, ps_proj, ypool)

    # Emission order tuned for overlap: get the first scores to ScalarE as
    # early as possible (exp is the bottleneck engine), slot the v
    # projection behind the first score batch, and interleave the next
    # pair's q/k projection into the middle of the current pair's attention.
    emit_qk_proj(0)
    spans = [[0, 1, 2], [3, 4, 5], [6, 7]]
    for m in range(MT):
        if m + 1 < MT:
            emit_qk_alloc(m + 1)
        for qc in range(4):
            if m == 0 and qc == 0:
                # one v-projection chain between successive score units so
                # the first AV can start as soon as possible after (0,0)
                E = emit_scores(m, qc, per_kt=lambda kt: emit_v_chain(kt))
            else:
                E = emit_scores(m, qc)
            if m == 0 and qc < 3:
                # deferred q-chunk projections for m=0 (chain 2=q1,4=q2,6=q3)
                emit_qk_chains(0, [2 * (qc + 1)])
            if qc < 3 and m + 1 < MT:
                emit_qk_chains(m + 1, spans[qc])
            emit_av(m, qc, E)
            if m == MT - 1:
                emit_outproj_block(qc)


def _emit_outproj_block_impl(nc, qb, valsn, wo, y_d, ps_proj, ypool):
    f32 = mybir.dt.float32
    for st in range(4 * qb, 4 * qb + 4):
        ssl = slice(st * 128, (st + 1) * 128)
        for oc in range(2):
            osl = slice(oc * 512, (oc + 1) * 512)
            p = ps_proj.tile([128, 512], f32, tag="pp", name="pop")
            for t in range(MT):
                nc.tensor.matmul(
                    p, valsn[t][:, ssl], wo[t][:, osl],
                    start=(t == 0), stop=(t == MT - 1),
                )
            ys = ypool.tile([128, 512], f32, tag="y", name="ys")
            nc.vector.tensor_copy(ys, p)
            nc.sync.dma_start(out=y_d[ssl, osl], in_=ys)


def _prep_core_inputs(x, Wq, bq, Wk, bk, Wv, bv, Wo):
    """Host-side shard prep. Returns list of per-core input dicts."""
    pnp = _NP[PROJ_DT]
    vnp = _NP[VO_DT]
    wq_s = (Wq * SCALE).astype(pnp)
    wk_s = Wk.astype(pnp)
    wv_s = Wv.astype(pnp)
    in_maps = []
    for c in range(NCORES):
        b = c // 2
        hh = c % 2
        cols = slice(hh * DH, (hh + 1) * DH)
        in_maps.append({
            "xt": np.ascontiguousarray(x[b].T).astype(pnp),
            "wq": np.ascontiguousarray(wq_s[:, cols]),
            "wk": np.ascontiguousarray(wk_s[:, cols]),
            "wv": np.ascontiguousarray(wv_s[:, cols]),
            "wo": np.ascontiguousarray(Wo[cols, :]).astype(vnp),
        })
    return in_maps


def _numpy_mha(x, Wq, bq, Wk, bk, Wv, bv, Wo, bo):
    y = np.empty((B, S, D), dtype=np.float32)
    for b in range(B):
        q = (x[b] @ Wq + bq).reshape(S, H, Hd).transpose(1, 0, 2)
        k = (x[b] @ Wk + bk).reshape(S, H, Hd).transpose(1, 0, 2)
        v = (x[b] @ Wv + bv).reshape(S, H, Hd).transpose(1, 0, 2)
        vals = np.empty((H, S, Hd), dtype=np.float32)
        for h in range(H):
            lg = (q[h] @ k[h].T) * SCALE
            lg -= lg.max(axis=-1, keepdims=True)
            e = np.exp(lg)
            vals[h] = (e @ v[h]) / e.sum(axis=-1, keepdims=True)
        y[b] = vals.transpose(1, 0, 2).reshape(S, D) @ Wo + bo
    return y


def kernel(x, Wq, bq, Wk, bk, Wv, bv, Wo, bo):
    x = np.asarray(x, dtype=np.float32)
    Wq = np.asarray(Wq, dtype=np.float32)
    Wk = np.asarray(Wk, dtype=np.float32)
    Wv = np.asarray(Wv, dtype=np.float32)
    Wo = np.asarray(Wo, dtype=np.float32)
    bq = np.asarray(bq, dtype=np.float32)
    bk = np.asarray(bk, dtype=np.float32)
    bv = np.asarray(bv, dtype=np.float32)
    bo = np.asarray(bo, dtype=np.float32)
    if max(np.abs(bq).max(), np.abs(bk).max(), np.abs(bv).max()) != 0:
        # The reference always uses zero q/k/v biases; keep a host fallback
        # for generality rather than failing.
        return _numpy_mha(x, Wq, bq, Wk, bk, Wv, bv, Wo, bo)

    if "nc" not in _CACHE:
        _CACHE["nc"] = _build_program()
    nc = _CACHE["nc"]

    in_maps = _prep_core_inputs(x, Wq, bq, Wk, bk, Wv, bv, Wo)
    res = run_bass_kernel_spmd(nc, in_maps, list(range(NCORES)))

    y = np.empty((B, S, D), dtype=np.float32)
    for b in range(B):
        y[b] = res.results[2 * b]["y"] + res.results[2 * b + 1]["y"]
    y += bo[None, None, :]
    return y



# revision 24
# speedup vs baseline: 1.3643x; 1.2393x over previous
"""Multi-head self-attention (B=4, S=2048, D=1024, H=16) on 8 Trainium2 cores.

Sharding: core c handles batch c//2 and head-half c%2 (8 heads = 512 dims).
Each core computes q/k/v projections for its heads, attention, and a partial
output projection (contraction over its 512 head-dims). Host sums the two
partials per batch and adds bo.

Schedule (HW-A/B-tested against the previous versions):
- qk_proj(0) emits k-chains + the first q-chunk chain first; scores (and
  ScalarE exp) start after 2 chains instead of 8. q1..q3 stream later.
- The two head-par AV chains run kt-interleaved on the two AV PSUM banks
  (no same-bank back-to-back accumulation); a single fast DVE copy drains
  each bank (released ~1.4us sooner for the next AV pair) and the
  normalization (reciprocal+broadcast+mul) runs SBUF-side. No cross-bank
  merge adds.
Earlier findings kept:
- Coarse 3D prologue DMAs in compute-priority order (x + m0 columns of
  wq/wk first): the first projection chain starts ~9us earlier.
- q/k projection chains for head-pair m+1 spread 3/3/2 over the qc=0..2
  windows of m instead of one lump at qc==2 (no PE burst at m boundaries).
- v-projection chains interleaved one-per-score-unit inside (0,0) so the
  first AV chain is unblocked as early as possible; epool 25 bufs of
  backlog keep ScalarE fed across that window.
- m=3 AV drains immediately followed by that q-block's output projection,
  overlapping the output tail with the last attention windows.

Self-contained: hardcodes all shapes; only needs the environment-provided
concourse libraries.
"""
import os
import sys

for _p in ("/opt/trn_rl_repo", "/root/.axon_site/_ro/trn_rl_repo"):
    if os.path.isdir(_p) and _p not in sys.path:
        sys.path.insert(0, _p)

import numpy as np
import ml_dtypes

import concourse.bass as bass  # noqa: F401
from concourse import bacc
import concourse.mybir as mybir
import concourse.tile as tile
from concourse.bass_utils import run_bass_kernel_spmd

# problem constants
B, S, D, H, Hd = 4, 2048, 1024, 16, 64
NCORES = 8
HH = H // 2            # heads per core
DH = HH * Hd           # head dims per core = 512
SCALE = 1.0 / np.sqrt(Hd)

ST = S // 128          # 16 seq tiles
QC = S // 512          # 4 q chunks
KT = ST                # 16 k tiles
DT_IN = D // 128       # 8 input-dim tiles
MT = DH // 128         # 4 head-pair tiles

f32 = mybir.dt.float32
bf16 = mybir.dt.bfloat16
np_bf16 = ml_dtypes.bfloat16

# matmul input dtypes per stage (fp32 accumulation always).
# float16: 1 cy/row on PE like bf16, 8x finer mantissa. exp(logit) must stay
# below 65504 — max |logit| for randn-style inputs is ~9 (exp ~6.6e3), safe.
PROJ_DT = mybir.dt.float16    # xT / wq / wk / wv
QK_DT = mybir.dt.float16      # qT / kT (scores matmul operands)
E_DT = mybir.dt.float16       # exp outputs and v_aug (AV matmul operands)
VO_DT = mybir.dt.float16      # valsn / wo (output projection operands)
_NP = {
    mybir.dt.bfloat16: np_bf16,
    mybir.dt.float16: np.float16,
    mybir.dt.float32r: np.float32,
    mybir.dt.float32: np.float32,
}

_CACHE = {}

# timing-ablation knob: subset of {"scores", "exp", "av", "outproj", "norm"}
# suppresses those instruction groups (breaks semantics, timing only)
SKIP = set()


def _build_program(repeat=1):
    nc = bacc.Bacc("TRN2", target_bir_lowering=False, debug=False)

    xt_d = nc.declare_dram_parameter("xt", [D, S], PROJ_DT, isOutput=False)
    wq_d = nc.declare_dram_parameter("wq", [D, DH], PROJ_DT, isOutput=False)
    wk_d = nc.declare_dram_parameter("wk", [D, DH], PROJ_DT, isOutput=False)
    wv_d = nc.declare_dram_parameter("wv", [D, DH], PROJ_DT, isOutput=False)
    wo_d = nc.declare_dram_parameter("wo", [DH, D], VO_DT, isOutput=False)
    y_d = nc.declare_dram_parameter("y", [S, D], f32, isOutput=True)

    Exp = mybir.ActivationFunctionType.Exp

    with tile.TileContext(nc) as tc:
        with (
            tc.tile_pool(name="wpool", bufs=1) as wpool,
            tc.tile_pool(name="xpool", bufs=1) as xpool,
            tc.tile_pool(name="qkpool", bufs=1) as qkpool,
            tc.tile_pool(name="vpool", bufs=1) as vpool,
            tc.tile_pool(name="vnpool", bufs=1) as vnpool,
            tc.tile_pool(name="epool", bufs=29) as epool,
            tc.tile_pool(name="spool", bufs=2) as spool,
            tc.tile_pool(name="ypool", bufs=2) as ypool,
            tc.tile_pool(name="ps_proj", bufs=2, space="PSUM") as ps_proj,
            tc.tile_pool(name="ps_score", bufs=2, space="PSUM") as ps_score,
            tc.tile_pool(name="ps_av", bufs=2, space="PSUM") as ps_av_pool,
        ):
            # ---- load inputs to SBUF ----
            # Coarse 3D DMAs in compute-priority order: qk_proj(0) needs x
            # plus only the m0 columns of wq/wk, so those land first and the
            # first projection chain starts ~15us earlier than with uniform
            # per-tile loads.
            xt_big = []
            for j in range(4):
                tl = xpool.tile([128, 2 * S], PROJ_DT, tag=f"xtb{j}", name=f"xtb{j}")
                xt_big.append(tl)
            wbig = {}
            for nm in ("wq", "wk", "wv"):
                wbig[nm] = [
                    wpool.tile([128, 4 * DH], PROJ_DT, tag=f"{nm}b{j}", name=f"{nm}b{j}")
                    for j in range(2)
                ]
            wo_big = wpool.tile([128, 4 * D], VO_DT, tag="wob", name="wob")

            def dma_x(j):
                src = xt_d[j * 256:(j + 1) * 256, :].rearrange(
                    "(g p) s -> p g s", p=128)
                nc.sync.dma_start(
                    out=xt_big[j].rearrange("p (g s) -> p g s", g=2), in_=src)

            def dma_w(nm, dram, j, cols=slice(0, DH)):
                src = dram[j * 512:(j + 1) * 512, cols].rearrange(
                    "(g p) c -> p g c", p=128)
                nc.sync.dma_start(
                    out=wbig[nm][j].rearrange(
                        "p (g c) -> p g c", g=4)[:, :, cols], in_=src)

            m0, mrest = slice(0, 128), slice(128, DH)
            dma_x(0)
            # PE clock warm-up: ~3.4us of dummy matmuls (no data deps) during
            # the input-DMA wait trips the HAM activity window, so the real
            # projection chains start at full clock. Strictly shorter than
            # the minimum x-DMA time, so it never delays real work.
            warm_sb = xpool.tile([128, 640], PROJ_DT, tag="warm", name="warm")
            nc.vector.memset(warm_sb, 0.0)
            warm_ps = ps_score.tile([128, 1024], f32, tag="score", name="warmps")
            for _ in range(8):
                nc.tensor.matmul(
                    warm_ps[:, 0:512], warm_sb[:, 0:128], warm_sb[:, 128:640],
                    start=True, stop=True,
                )
            dma_w("wq", wq_d, 0, m0)
            dma_w("wk", wk_d, 0, m0)
            dma_x(1)
            dma_w("wq", wq_d, 1, m0)
            dma_w("wk", wk_d, 1, m0)
            dma_x(2)
            dma_x(3)
            dma_w("wv", wv_d, 0)
            dma_w("wv", wv_d, 1)
            dma_w("wq", wq_d, 0, mrest)
            dma_w("wk", wk_d, 0, mrest)
            dma_w("wq", wq_d, 1, mrest)
            dma_w("wk", wk_d, 1, mrest)
            nc.sync.dma_start(
                out=wo_big.rearrange("p (g c) -> p g c", g=4),
                in_=wo_d.rearrange("(g p) c -> p g c", p=128),
            )

            xt = [xt_big[t // 2][:, (t % 2) * S:(t % 2 + 1) * S] for t in range(DT_IN)]
            wq = [wbig["wq"][t // 4][:, (t % 4) * DH:(t % 4 + 1) * DH] for t in range(DT_IN)]
            wk = [wbig["wk"][t // 4][:, (t % 4) * DH:(t % 4 + 1) * DH] for t in range(DT_IN)]
            wv = [wbig["wv"][t // 4][:, (t % 4) * DH:(t % 4 + 1) * DH] for t in range(DT_IN)]
            wo = [wo_big[:, t * D:(t + 1) * D] for t in range(MT)]

            # ---- compute body (repeatable for timing runs) ----
            for _rep in range(repeat):
                _emit_body(nc, tc, xt, wq, wk, wv, wo, y_d,
                           vpool, vnpool, qkpool, epool, spool, ypool,
                           ps_proj, ps_score, ps_av_pool,
                           warm=(warm_sb, warm_ps) if _rep == 0 else None)

    nc.compile()
    return nc


def _emit_body(nc, tc, xt, wq, wk, wv, wo, y_d,
               vpool, vnpool, qkpool, epool, spool, ypool,
               ps_proj, ps_score, ps_av_pool, warm=None):
    f32 = mybir.dt.float32
    Exp = mybir.ActivationFunctionType.Exp

    v_sb = []
    for st in range(ST):
        v_sb.append(vpool.tile([128, HH * 66], E_DT, tag=f"v{st}", name=f"v{st}"))

    def emit_v_chain(st):
        # v_sb[st] = [128 seq, 8 heads x (64 v | 1 one | pad)]
        vt = v_sb[st]
        p = ps_proj.tile([128, DH], f32, tag="pp", name="pvp")
        for t in range(DT_IN):
            nc.tensor.matmul(
                p, xt[t][:, st * 128:(st + 1) * 128], wv[t],
                start=(t == 0), stop=(t == DT_IN - 1),
            )
        nc.vector.tensor_copy(
            vt.rearrange("p (h w) -> p h w", w=66)[:, :, 0:64],
            p.rearrange("p (h w) -> p h w", w=64),
        )
        nc.vector.memset(vt.rearrange("p (h w) -> p h w", w=66)[:, :, 64:65], 1.0)

    qk = {}

    def emit_qk_alloc(m):
        qT = qkpool.tile([128, S], QK_DT, tag=f"qT{m}", name=f"qT{m}")
        kT = qkpool.tile([128, S], QK_DT, tag=f"kT{m}", name=f"kT{m}")
        qk[m] = (qT, kT)

    warm_left = [16] if warm is not None else [0]

    def emit_qk_chains(m, cs):
        # chain index c in 0..7: c%2 selects q/k, c//2 selects the q-chunk
        qT, kT = qk[m]
        for c in cs:
            dst = qT if c % 2 == 0 else kT
            w = wq if c % 2 == 0 else wk
            nck = c // 2
            p = ps_proj.tile([128, 512], f32, tag="pp", name="pqk")
            for t in range(DT_IN):
                nc.tensor.matmul(
                    p,
                    w[t][:, m * 128:(m + 1) * 128],
                    xt[t][:, nck * 512:(nck + 1) * 512],
                    start=(t == 0), stop=(t == DT_IN - 1),
                )
                if warm_left[0] > 0:
                    # keep the PE busy through the input-DMA stalls so the
                    # HAM clock window never sees a >3.4us idle gap
                    warm_sb, warm_ps = warm
                    nc.tensor.matmul(
                        warm_ps[:, 0:512], warm_sb[:, 0:128],
                        warm_sb[:, 128:640], start=True, stop=True,
                    )
                    warm_left[0] -= 1
            nc.vector.tensor_copy(dst[:, nck * 512:(nck + 1) * 512], p)

    def emit_qk_proj(m):
        # scores(0,0) needs all of kT but only the first q-chunk of qT:
        # emit k0+q0 first (first exp possible after 2 chains), then the
        # remaining k chains; q1..q3 are deferred into later windows.
        emit_qk_alloc(m)
        emit_qk_chains(m, [1, 0, 3, 5, 7])

    valsn = []
    for m in range(MT):
        vn = vnpool.tile([128, S], VO_DT, tag=f"vn{m}")
        valsn.append(vn)

    def emit_scores(m, qc, per_kt=None):
        qT, kT = qk[m]
        qsl = slice(qc * 512, (qc + 1) * 512)
        E = []
        for kt in range(KT):
            if per_kt is not None:
                per_kt(kt)
            ksl = slice(kt * 128, (kt + 1) * 128)
            ps = ps_score.tile([128, 1024], f32, tag="score")
            if "scores" not in SKIP:
                nc.tensor.matmul(
                    ps[:, 0:512], kT[0:64, ksl], qT[0:64, qsl],
                    start=True, stop=True,
                )
                nc.tensor.matmul(
                    ps[:, 512:1024], kT[64:128, ksl], qT[64:128, qsl],
                    start=True, stop=True,
                )
            else:
                nc.tensor.matmul(
                    ps[0:8, 0:8], kT[0:64, kt * 128:kt * 128 + 8], qT[0:64, qsl][:, 0:8],
                    start=True, stop=True,
                )
                nc.tensor.matmul(
                    ps[0:8, 512:520], kT[64:128, kt * 128:kt * 128 + 8], qT[64:128, qsl][:, 0:8],
                    start=True, stop=True,
                )
            e = epool.tile([128, 1024], E_DT, tag="e")
            if "exp" not in SKIP:
                nc.scalar.activation(e, ps, Exp)
            else:
                nc.scalar.activation(e[0:8, 0:8], ps[0:8, 0:8], Exp)
            E.append(e)
        return E

    def emit_av(m, qc, E):
        # The two head-par chains run kt-interleaved on the two AV banks
        # (accumulating matmuls never hit the same bank twice in a row),
        # and normalization reads each PSUM bank directly: no cross-bank
        # merge adds on DVE.
        qsl = slice(qc * 512, (qc + 1) * 512)
        pav = [ps_av_pool.tile([65, 512], f32, tag="av", name="pav")
               for _ in range(2)]
        for kt in range(KT):
            for par in range(2):
                h = 2 * m + par
                nc.tensor.matmul(
                    pav[par],
                    v_sb[kt][:, h * 66:h * 66 + 65],
                    E[kt][:, par * 512:(par + 1) * 512],
                    start=(kt == 0), stop=(kt == KT - 1),
                )
        last = (m == MT - 1 and qc == QC - 1)
        for par in range(2):
            psl = slice(par * 64, (par + 1) * 64)
            if last:
                # final pair: no later AV needs these banks — normalize
                # straight from PSUM to cut latency to the last outproj block
                src_t = pav[par]
            else:
                # single fast copy releases the PSUM bank for the next AV
                # pair; the rest of the normalization runs SBUF-side
                src_t = spool.tile([65, 512], f32, tag="tmp", name="tmp")
                nc.vector.tensor_copy(src_t, pav[par])
            srow0 = spool.tile([1, 512], f32, tag="srow0", name="srow0")
            # denominators are sums of 2048 positive exps (1e2..5e4): the
            # ~51-ULP fast reciprocal is exact enough and ~5x faster, off
            # the AV->valsn->outproj latency chain
            nc.vector.reciprocal_approx_fast(srow0, src_t[64:65, :])
            bc = spool.tile([128, 512], f32, tag="bc", name="bc")
            nc.gpsimd.partition_broadcast(bc, srow0)
            if par == 0:
                nc.vector.tensor_mul(
                    valsn[m][0:64, qsl], src_t[0:64, :], bc[0:64, :])
            else:
                avs = spool.tile([128, 512], f32, tag="avs", name="avs")
                nc.vector.tensor_copy(avs[psl, :], src_t[0:64, :])
                nc.vector.tensor_mul(valsn[m][psl, qsl], avs[psl, :], bc[psl, :])

    def emit_outproj_block(qb):
        _emit_outproj_block_impl(nc, qb, valsn, wo, y_d, ps_proj, ypool)

    # Emission order tuned for overlap: get the first scores to ScalarE as
    # early as possible (exp is the bottleneck engine), slot the v
    # projection behind the first score batch, and interleave the next
    # pair's q/k projection into the middle of the current pair's attention.
    emit_qk_proj(0)
    spans = [[0, 1, 2], [3, 4, 5], [6, 7]]
    for m in range(MT):
        if m + 1 < MT:
            emit_qk_alloc(m + 1)
        for qc in range(4):
            if m == 0 and qc == 0:
                # one v-projection chain between successive score units so
                # the first AV can start as soon as possible after (0,0)
                E = emit_scores(m, qc, per_kt=lambda kt: emit_v_chain(kt))
            else:
                E = emit_scores(m, qc)
            if m == 0 and qc < 3:
                # deferred q-chunk projections for m=0 (chain 2=q1,4=q2,6=q3)
                emit_qk_chains(0, [2 * (qc + 1)])
            if qc < 3 and m + 1 < MT:
                emit_qk_chains(m + 1, spans[qc])
            emit_av(m, qc, E)
            if m == MT - 1:
                emit_outproj_block(qc)


def _emit_outproj_block_impl(nc, qb, valsn, wo, y_d, ps_proj, ypool):
    f32 = mybir.dt.float32
    for st in range(4 * qb, 4 * qb + 4):
        ssl = slice(st * 128, (st + 1) * 128)
        for oc in range(2):
            osl = slice(oc * 512, (oc + 1) * 512)
            p = ps_proj.tile([128, 512], f32, tag="pp", name="pop")
            for t in range(MT):
                nc.tensor.matmul(
                    p, valsn[t][:, ssl], wo[t][:, osl],
                    start=(t == 0), stop=(t == MT - 1),
                )
            ys = ypool.tile([128, 512], f32, tag="y", name="ys")
            nc.vector.tensor_copy(ys, p)
            nc.sync.dma_start(out=y_d[ssl, osl], in_=ys)


def _prep_core_inputs(x, Wq, bq, Wk, bk, Wv, bv, Wo):
    """Host-side shard prep. Returns list of per-core input dicts."""
    pnp = _NP[PROJ_DT]
    vnp = _NP[VO_DT]
    wq_s = (Wq * SCALE).astype(pnp)
    wk_s = Wk.astype(pnp)
    wv_s = Wv.astype(pnp)
    in_maps = []
    for c in range(NCORES):
        b = c // 2
        hh = c % 2
        cols = slice(hh * DH, (hh + 1) * DH)
        in_maps.append({
            "xt": np.ascontiguousarray(x[b].T).astype(pnp),
            "wq": np.ascontiguousarray(wq_s[:, cols]),
            "wk": np.ascontiguousarray(wk_s[:, cols]),
            "wv": np.ascontiguousarray(wv_s[:, cols]),
            "wo": np.ascontiguousarray(Wo[cols, :]).astype(vnp),
        })
    return in_maps


def _numpy_mha(x, Wq, bq, Wk, bk, Wv, bv, Wo, bo):
    y = np.empty((B, S, D), dtype=np.float32)
    for b in range(B):
        q = (x[b] @ Wq + bq).reshape(S, H, Hd).transpose(1, 0, 2)
        k = (x[b] @ Wk + bk).reshape(S, H, Hd).transpose(1, 0, 2)
        v = (x[b] @ Wv + bv).reshape(S, H, Hd).transpose(1, 0, 2)
        vals = np.empty((H, S, Hd), dtype=np.float32)
        for h in range(H):
            lg = (q[h] @ k[h].T) * SCALE
            lg -= lg.max(axis=-1, keepdims=True)
            e = np.exp(lg)
            vals[h] = (e @ v[h]) / e.sum(axis=-1, keepdims=True)
        y[b] = vals.transpose(1, 0, 2).reshape(S, D) @ Wo + bo
    return y


def kernel(x, Wq, bq, Wk, bk, Wv, bv, Wo, bo):
    x = np.asarray(x, dtype=np.float32)
    Wq = np.asarray(Wq, dtype=np.float32)
    Wk = np.asarray(Wk, dtype=np.float32)
    Wv = np.asarray(Wv, dtype=np.float32)
    Wo = np.asarray(Wo, dtype=np.float32)
    bq = np.asarray(bq, dtype=np.float32)
    bk = np.asarray(bk, dtype=np.float32)
    bv = np.asarray(bv, dtype=np.float32)
    bo = np.asarray(bo, dtype=np.float32)
    if max(np.abs(bq).max(), np.abs(bk).max(), np.abs(bv).max()) != 0:
        # The reference always uses zero q/k/v biases; keep a host fallback
        # for generality rather than failing.
        return _numpy_mha(x, Wq, bq, Wk, bk, Wv, bv, Wo, bo)

    if "nc" not in _CACHE:
        _CACHE["nc"] = _build_program()
    nc = _CACHE["nc"]

    in_maps = _prep_core_inputs(x, Wq, bq, Wk, bk, Wv, bv, Wo)
    res = run_bass_kernel_spmd(nc, in_maps, list(range(NCORES)))

    y = np.empty((B, S, D), dtype=np.float32)
    for b in range(B):
        y[b] = res.results[2 * b]["y"] + res.results[2 * b + 1]["y"]
    y += bo[None, None, :]
    return y

